# revision 2
# baseline (speedup 1.0000x reference)
"""Fused ConvTranspose3d(stride2,pad1) + scale + AvgPool3d(2) + bias kernel, TRN2.

Math: transposed conv (K=3,S=2,P=1) + AvgPool(2) collapse to a stride-1 VALID
2x2x2-tap conv: per-dim taps tap0 = W[1]+W[2], tap1 = W[0];
z = conv(x, V)*(s1*s2/8) + (conv_bias*s1 + bias)*s2.

V4 = bf16 "plan A": host replicates x 4x over the (d,h)-taps so one k=128
matmul per 512-position block covers all (a,b) taps with a single stationary
(no weight reloads beyond the fused per-mm load):
    k = 128 = 4 (d,h)-taps x 32 c_in   (shifted replicas, bf16, from DRAM)
    m = 128 = 2 w-taps x 64 c_out
    z[co,oh,ow] = ps[c0,(oh,ow)] + ps[c1,(oh,ow+1)] + beta
finalize: ACT drains c0-half + bias -> bf16, DVE adds the shifted c1 PSUM
half; output bf16, widened to f32 on host.
Data parallel: batch 16 -> 2 per core on 8 cores.
"""

import sys

if "/opt/trn_rl_repo" not in sys.path:
    sys.path.insert(0, "/opt/trn_rl_repo")

from contextlib import ExitStack

import numpy as np
import ml_dtypes

import concourse.bass as bass
import concourse.tile as tile
from concourse import mybir
from concourse.bass_utils import run_bass_kernel_spmd
from concourse.vector_clock import ScopedClock as _ScopedClock


def _patched_drain_and_barrier(self, tick_clock, wait_clock):
    nc = self.nc
    drain_inst = nc.sync.drain()
    wait_clock.add_sem_waits(
        drain_inst.ins, _ScopedClock({None: tick_clock.global_clock})
    )
    waits = list(drain_inst.ins.sync_info.on_wait)
    if len(waits) > 1:
        drain_inst.ins.sync_info.on_wait = waits[:1]
        for w in waits[1:]:
            n = nc.sync.nop(nofuse=True)
            n.ins.sync_info = mybir.SyncInfo(on_wait=[w], on_update=[])
    nc.all_engine_barrier()
    assert self.sems is not None
    popped = nc._tile_sem_poison_stack.pop()
    assert popped is self._sem_poison
    nc.clear_and_free_semaphores(list(self.sems.allocated().values()))
    nc.all_engine_barrier()


tile.TileContext._drain_and_barrier = _patched_drain_and_barrier


def _legalize_sync_waits(nc, max_waits=1):
    """walrus codegen allows very few sync-waits per instruction; move excess
    waits onto nop carriers on the same engine right before the instruction."""
    for fn in nc.m.functions:
        for bb in fn.blocks:
            new_insts = []
            changed = False
            for inst in bb.instructions:
                si = getattr(inst, "sync_info", None)
                if si is not None and si.on_wait and len(si.on_wait) > max_waits:
                    waits = list(si.on_wait)
                    si.on_wait = waits[-max_waits:]
                    extra = waits[:-max_waits]
                    for i in range(0, len(extra), max_waits):
                        nop = mybir.InstNoOp(
                            name=nc.get_next_instruction_name(),
                            engine=inst.engine,
                            sync_info=mybir.SyncInfo(
                                on_wait=extra[i : i + max_waits], on_update=[]
                            ),
                            bass_nofuse=True,
                        )
                        new_insts.append(nop)
                    changed = True
                new_insts.append(inst)
            if changed:
                bb.instructions[:] = new_insts

N, C_IN, C_OUT = 16, 32, 64
D = H = W = 32
OD = OH = OW = 31
NCORES = 8
NB = N // NCORES
PLANE = H * W  # 1024
VOL = D * PLANE  # 32768
ZPLANE = OH * OW  # 961
ZVOL = OD * ZPLANE  # 29791

_CHUNKS = [(0, 8), (8, 8), (16, 8), (24, 7)]
_ROWBLOCKS = [(0, 16), (16, 15)]


def _build_program(legalize=True):
    nc = bass.Bass(
        "TRN2", target_bir_lowering=False, debug=False, num_swdge_queues=4
    )
    f32 = mybir.dt.float32
    bf16 = mybir.dt.bfloat16
    # x pre-replicated on host: partition dim = (2a+b)*32+ci holding
    # x[ci, f + a*PLANE + b*W], bf16
    x_ap = nc.dram_tensor("x", [NB, 128, VOL], bf16, kind="ExternalInput").ap()
    w_ap = nc.dram_tensor("wpack", [128, 128], bf16, kind="ExternalInput").ap()
    b_ap = nc.dram_tensor("beta", [C_OUT, 1], f32, kind="ExternalInput").ap()
    b2_ap = nc.dram_tensor("beta128", [128, 1], f32, kind="ExternalInput").ap()
    z_ap = nc.dram_tensor("z", [NB, C_OUT, ZVOL], bf16, kind="ExternalOutput").ap()

    ident = mybir.ActivationFunctionType.Identity
    add = mybir.AluOpType.add

    with tile.TileContext(nc) as tc, ExitStack() as ctx:
        wpool = ctx.enter_context(tc.tile_pool(name="w", bufs=1))
        x4pool = ctx.enter_context(tc.tile_pool(name="x4", bufs=3))
        pspool = ctx.enter_context(tc.tile_pool(name="ps", bufs=4, space="PSUM"))
        zcpool = ctx.enter_context(tc.tile_pool(name="zc", bufs=10))
        ogpool = ctx.enter_context(tc.tile_pool(name="og", bufs=3))

        wt = wpool.tile([128, 128], bf16)
        nc.sync.dma_start(wt[:], w_ap[:])
        bt = wpool.tile([C_OUT, 1], f32)
        nc.sync.dma_start(bt[:], b_ap[:])
        bt2 = wpool.tile([128, 1], f32)
        nc.sync.dma_start(bt2[:], b2_ap[:])

        fin_i = 0
        for b in range(NB):
            for p0, nsl in _CHUNKS:
                ch_need = (nsl - 1) * PLANE + 992 + 64
                x4 = x4pool.tile([128, ch_need], bf16, tag="x4")
                nc.sync.dma_start(
                    x4[:], x_ap[b, :, p0 * PLANE : p0 * PLANE + ch_need]
                )
                og = ogpool.tile([C_OUT, nsl * ZPLANE], bf16, tag="og")
                for l in range(nsl):
                    dst = og[:, l * ZPLANE : (l + 1) * ZPLANE].rearrange(
                        "p (a b) -> p a b", b=OW
                    )
                    fin_i += 1
                    if fin_i % 3 == 0:
                        # m=64 drain-only slab: the w-tap is accumulated in
                        # PSUM by a second matmul at moving offset +1, so the
                        # finalize is a single biased drain (no DVE add).
                        ps = pspool.tile([C_OUT, OH, W], f32, tag="ps", name="ps")
                        for oh0, nrows in _ROWBLOCKS:
                            base = l * PLANE + oh0 * W
                            for c in (0, 1):
                                nc.tensor.matmul(
                                    ps[:, oh0 : oh0 + nrows, :],
                                    wt[:, c * C_OUT : (c + 1) * C_OUT],
                                    x4[:, base + c : base + c + nrows * W],
                                    start=c == 0,
                                    stop=c == 1,
                                )
                        if fin_i % 2 == 0:
                            nc.scalar.activation(
                                dst,
                                ps[:, :, 0:OW],
                                ident,
                                bias=bt[:, 0:1],
                                scale=1.0,
                            )
                        else:
                            nc.vector.tensor_scalar_add(
                                dst, ps[:, :, 0:OW], bt[:, 0:1]
                            )
                    else:
                        ps = pspool.tile([128, OH, W], f32, tag="ps", name="ps")
                        for oh0, nrows in _ROWBLOCKS:
                            base = l * PLANE + oh0 * W
                            nc.tensor.matmul(
                                ps[:, oh0 : oh0 + nrows, :],
                                wt[:],
                                x4[:, base : base + nrows * W],
                                start=True,
                                stop=True,
                            )
                        zc = zcpool.tile([C_OUT, OH, OW], bf16, tag="zc0")
                        nc.scalar.activation(
                            zc[:],
                            ps[0:C_OUT, :, 0:OW],
                            ident,
                            bias=bt[:, 0:1],
                            scale=1.0,
                        )
                        nc.vector.tensor_add(dst, zc[:], ps[C_OUT:128, :, 1:W])
                zbase = p0 * ZPLANE
                nc.scalar.dma_start(
                    z_ap[b, :, zbase : zbase + nsl * ZPLANE], og[:]
                )
    if legalize:
        _legalize_sync_waits(nc)
    return nc


def _host_prep(weight, conv_bias, bias, scale1, scale2):
    w = np.asarray(weight, dtype=np.float64)  # (C_IN, C_OUT, 3,3,3)
    s1 = float(np.asarray(scale1))
    s2 = float(np.asarray(scale2))
    taps = [[1, 2], [0]]  # tap0 = W[1]+W[2], tap1 = W[0]
    alpha = s1 * s2 / 8.0
    wpack = np.zeros((128, 128), dtype=np.float64)
    for a in range(2):
        for hb in range(2):
            t = 2 * a + hb
            for c in range(2):
                v = np.zeros((C_IN, C_OUT), dtype=np.float64)
                for kd in taps[a]:
                    for kh in taps[hb]:
                        for kw in taps[c]:
                            v += w[:, :, kd, kh, kw]
                wpack[t * C_IN : (t + 1) * C_IN, c * C_OUT : (c + 1) * C_OUT] = (
                    alpha * v
                )
    beta = (
        (np.asarray(conv_bias, dtype=np.float64).reshape(-1) * s1
         + np.asarray(bias, dtype=np.float64).reshape(-1))
        * s2
    ).astype(np.float32).reshape(C_OUT, 1)
    beta128 = np.zeros((128, 1), dtype=np.float32)
    beta128[:C_OUT] = beta
    return wpack.astype(ml_dtypes.bfloat16), beta, beta128


def kernel(x, weight, conv_bias, bias, scale1, scale2, _trace=False):
    x = np.asarray(x, dtype=np.float32)
    wpack, beta, beta128 = _host_prep(weight, conv_bias, bias, scale1, scale2)

    # host-side tap replication: xrep[n, (2a+b)*32+ci, f] = x[n, ci, f+a*PLANE+b*W]
    xf = x.reshape(N, C_IN, VOL)
    xrep = np.zeros((N, 4, C_IN, VOL), dtype=ml_dtypes.bfloat16)
    for t, s in enumerate((0, W, PLANE, PLANE + W)):
        xrep[:, t, :, 0 : VOL - s] = xf[:, :, s:VOL]
    xrep = xrep.reshape(N, 128, VOL)

    nc = _build_program()
    in_maps = []
    for core in range(NCORES):
        xs = xrep[core * NB : (core + 1) * NB]
        in_maps.append(
            {"x": np.ascontiguousarray(xs), "wpack": wpack, "beta": beta,
             "beta128": beta128}
        )
    res = run_bass_kernel_spmd(
        nc, in_maps, core_ids=list(range(NCORES)), trace=_trace
    )
    z = np.empty((N, C_OUT, OD, OH, OW), dtype=np.float32)
    for core in range(NCORES):
        z[core * NB : (core + 1) * NB] = (
            res.results[core]["z"].astype(np.float32).reshape(NB, C_OUT, OD, OH, OW)
        )
    if _trace:
        return z, res
    return z
